# revision 44
# baseline (speedup 1.0000x reference)
"""Trainium2 Bass kernel for nn_EventPairCompositionModel.

Strategy (data-parallel over batch, 8 cores, B=512 -> 64 per core):
  - Host compacts the f32 table per core to the ~24K unique rows its shard
    touches.  Embeddings are stored fp8 (e4m3, x16 scaled): elems 0..255 in a
    256B-row table fetched on-device with SWDGE transpose dma_gather
    (row-rate-bound, so fp8 minimizes bytes); elems 256..299 ride in a small
    host-packed per-(b,n) tail table loaded with one plain contiguous DMA.
  - The 16-bit-granularity transpose gather lands fp8 element pairs
    (2p, 2p+1) in 16-bit cells -> exactly the [K=128, 2] operand layout the
    PE's fp8 DoubleRow perf mode wants: the shared arg-composition MLP runs
    at 2x bf16 throughput, 5 DoubleRow passes for the K=1200 first layer
    (4 component mains + 1 combined tail) with zero pad waste.
  - All activations are fp8 with scales folded into activation scale/bias
    (s1 = 8*h1, s2 = 8*h2), so MLP2, cosine numerators (stationary = event
    column) and |c|^2 ones-reductions are DoubleRow as well.
  - relu1 runs on Scalar, relu2 on Vector (engine balance); per-group [1,512]
    numerator/norm rows go SBUF->DRAM scratch and are re-read [64b, 128n].
    KNRM pooling folds (t-mu)^2/(2s^2) into Square activations (present in
    every ACT table -> no table swaps) plus one batched Exp and one
    reduce_sum; the final score math runs 64 lanes wide.
  - If a shard ever touches >32767 unique rows (can't happen for random
    inputs), falls back to exact host math.
All 8 cores run the identical program on their own batch shard (SPMD, no
collectives); host concatenates the 8 (64,1) outputs.
"""

import numpy as np
import ml_dtypes

import concourse.bacc as bacc
import concourse.tile as tile
import concourse.mybir as mybir
from concourse.bass_utils import run_bass_kernel_spmd
from concourse import library_config

F32 = mybir.dt.float32
BF16 = mybir.dt.bfloat16
F8 = mybir.dt.float8e4
I16 = mybir.dt.int16
AF = mybir.ActivationFunctionType
DR = mybir.MatmulPerfMode.DoubleRow
MUL = mybir.AluOpType.mult
ADD = mybir.AluOpType.add
MAXOP = mybir.AluOpType.max

# Problem shapes (hardcoded per spec)
B, N, C, E = 512, 128, 4, 300
V = 50000
H1, H2 = 512, 256
NF, NK = 8, 11
NCORES = 8
BC = B // NCORES          # 64 batches per core
EM = 256                  # fp8 main row elems (0..255)
ET = E - EM               # 44 tail elems per component
KT = C * ET               # 176 combined tail K-rows
CT = 32768                # compact table rows (int16-indexable)
GROUPS = (BC * N) // 512  # 16 groups of 512 (b,n) pairs
SUBT = 4                  # 128-bn subtiles per group (one batch b each)
BN = BC * N               # 8192 (b,n) pairs per core
XS = 16.0                 # fp8 table scale
HS = 8.0                  # fp8 hidden-activation scale

MUS = [1.0, 0.9, 0.7, 0.5, 0.3, 0.1, -0.1, -0.3, -0.5, -0.7, -0.9]
SIGMAS = [1e-3] + [0.1] * 10

_PROGRAM_CACHE = {}


def _build_fast():
    if "fast" in _PROGRAM_CACHE:
        return _PROGRAM_CACHE["fast"]

    nc = bacc.Bacc("TRN2", target_bir_lowering=False, debug=False, num_swdge_queues=4)

    # ---- DRAM I/O ----
    ctab = nc.dram_tensor("ctab", (CT, EM), F8, kind="ExternalInput")
    tailc = nc.dram_tensor("tailc", (128, 2 * BN), F8, kind="ExternalInput")
    taile = nc.dram_tensor("taile", (128, 2 * 128), F8, kind="ExternalInput")
    cidx = nc.dram_tensor("cidx", (128, GROUPS * 128), I16, kind="ExternalInput")
    eidx = nc.dram_tensor("eidx", (128, 32), I16, kind="ExternalInput")
    # weights: K-order-matched fp8 layouts (see _prep_fast_consts)
    w1m = nc.dram_tensor("w1m", (128, C * 2 * H1), F8, kind="ExternalInput")
    w1t = nc.dram_tensor("w1t", (128, 2 * H1), F8, kind="ExternalInput")
    w2q = nc.dram_tensor("w2q", (128, 4 * H2), F8, kind="ExternalInput")
    wvm = nc.dram_tensor("wvm", (128, 2 * 16), F8, kind="ExternalInput")
    wvt = nc.dram_tensor("wvt", (128, 2 * 16), F8, kind="ExternalInput")
    b1d = nc.dram_tensor("b1d", (128, 4), F32, kind="ExternalInput")   # 8*b1
    b2d = nc.dram_tensor("b2d", (128, 2), F32, kind="ExternalInput")   # 8*b2
    bvd = nc.dram_tensor("bvd", (9, 1), F32, kind="ExternalInput")
    wcb = nc.dram_tensor("wcb", (BC, 48), F32, kind="ExternalInput")   # per-b Wc row
    wkb = nc.dram_tensor("wkb", (BC, NK), F32, kind="ExternalInput")   # 0.01*Wc_kp
    bcd = nc.dram_tensor("bcd", (BC, 1), F32, kind="ExternalInput")    # -bc
    ndsq = nc.dram_tensor("ndsq", (9, BC), F32, kind="ExternalInput")  # -(d*d).T
    featd = nc.dram_tensor("featd", (BC, NF), F32, kind="ExternalInput")
    kpb = nc.dram_tensor("kpb", (BC, NK), F32, kind="ExternalInput")   # -mu_k/(sk*sqrt2)
    out_d = nc.dram_tensor("out", (BC, 1), F32, kind="ExternalOutput")

    with tile.TileContext(nc) as tc:
        with (
            tc.tile_pool(name="consts", bufs=1) as cpool,
            tc.tile_pool(name="xt", bufs=16) as xtpool,
            tc.tile_pool(name="s1", bufs=3) as s1pool,
            tc.tile_pool(name="s2", bufs=3) as s2pool,
            tc.tile_pool(name="csq", bufs=3) as csqpool,
            tc.tile_pool(name="small", bufs=2) as smpool,
            tc.tile_pool(name="pm1", bufs=3, space="PSUM") as pm1,
            tc.tile_pool(name="pm2", bufs=1, space="PSUM") as pm2,
            tc.tile_pool(name="ptn", bufs=1, space="PSUM") as ptn,
            tc.tile_pool(name="pmisc", bufs=2, space="PSUM") as pmisc,
            tc.tile_pool(name="dsc", bufs=1, space="DRAM") as dpool,
        ):
            nc.gpsimd.load_library(library_config.mlp)
            # ---- index loads first so gathers can start ASAP ----
            eidx_s = cpool.tile([128, 32], I16)
            nc.sync.dma_start(eidx_s[:], eidx.ap())
            cidx_s = cpool.tile([128, GROUPS * 128], I16)
            nc.sync.dma_start(cidx_s[:], cidx.ap())

            # ---- event gather (512 idx = (c, b): 64 real + 64 junk b) ----
            xe = cpool.tile([128, 2 * 512], F8)
            nc.gpsimd.dma_gather(
                out_ap=xe[:].rearrange("p (j i) -> p j i", j=2),
                in_ap=ctab.ap(),
                idxs_ap=eidx_s[:],
                num_idxs=512,
                num_idxs_reg=512,
                elem_size=EM,
                transpose=True,
            )

            # ---- context gathers (16 groups x 4 subtiles x 512 idx) ----
            xts = []
            for g in range(GROUPS):
                xt = xtpool.tile([128, SUBT * 2 * 512], F8, tag="xt", name=f"xt_{g}")
                for s in range(SUBT):
                    nc.gpsimd.dma_gather(
                        out_ap=xt[:].rearrange(
                            "p (z j i) -> p z j i", z=SUBT, j=2
                        )[:, s, :, :],
                        in_ap=ctab.ap(),
                        idxs_ap=cidx_s[
                            :, 32 * (SUBT * g + s) : 32 * (SUBT * g + s + 1)
                        ],
                        num_idxs=512,
                        num_idxs_reg=512,
                        elem_size=EM,
                        transpose=True,
                    )
                xts.append(xt)

            # ---- tail tables / weights (scalar HWDGE queue, so the sync
            # queue only carries the small idx loads the gathers wait on) ----
            taile_s = cpool.tile([128, 2 * 128], F8)
            nc.scalar.dma_start(taile_s[:], taile.ap())

            # ---- weights / consts ----
            w1m_s = cpool.tile([128, C * 2 * H1], F8)
            nc.scalar.dma_start(w1m_s[:], w1m.ap())
            w1t_s = cpool.tile([128, 2 * H1], F8)
            nc.scalar.dma_start(w1t_s[:], w1t.ap())
            w2q_s = cpool.tile([128, 4 * H2], F8)
            nc.scalar.dma_start(w2q_s[:], w2q.ap())
            wvm_s = cpool.tile([128, 2 * 16], F8)
            nc.scalar.dma_start(wvm_s[:], wvm.ap())
            wvt_s = cpool.tile([128, 2 * 16], F8)
            nc.scalar.dma_start(wvt_s[:], wvt.ap())
            tailc_s = cpool.tile([128, 2 * BN], F8)
            nc.scalar.dma_start(tailc_s[:], tailc.ap())
            b1_s = cpool.tile([128, 4], F32)
            nc.sync.dma_start(b1_s[:], b1d.ap())
            b2_s = cpool.tile([128, 2], F32)
            nc.sync.dma_start(b2_s[:], b2d.ap())
            bv_s = cpool.tile([9, 1], F32)
            nc.sync.dma_start(bv_s[:], bvd.ap())
            wcb_s = cpool.tile([BC, 48], F32)
            nc.sync.dma_start(wcb_s[:], wcb.ap())
            wkb_s = cpool.tile([BC, NK], F32)
            nc.sync.dma_start(wkb_s[:], wkb.ap())
            bc_s = cpool.tile([BC, 1], F32)
            nc.sync.dma_start(bc_s[:], bcd.ap())
            ndsq_s = cpool.tile([9, BC], F32)
            nc.sync.dma_start(ndsq_s[:], ndsq.ap())
            featd_s = cpool.tile([BC, NF], F32)
            nc.sync.dma_start(featd_s[:], featd.ap())
            kpb_s = cpool.tile([BC, NK], F32)
            nc.sync.dma_start(kpb_s[:], kpb.ap())
            ones8 = cpool.tile([128, 32], F8)
            nc.vector.memset(ones8[:], 1.0)

            # DRAM scratch for numerator/norm rows (split per half so the
            # first half's read only depends on groups 0..7)
            trd = [dpool.tile([1, BN // 2], F32, name=f"trd{h}") for h in range(2)]
            ncd = [dpool.tile([1, BN // 2], F32, name=f"ncd{h}") for h in range(2)]

            # DoubleRow helpers ------------------------------------------
            # gathered fp8 layout: byte (p, f) = elem 2p + f%2 of row idx[f//2]
            def xmain(t, c):
                return t.rearrange("p (i q) -> p q i", q=2)[:, :, 128 * c : 128 * (c + 1)]

            def w1_main(c, m):
                return w1m_s[:].rearrange("p (c q m) -> p c q m", c=C, q=2)[
                    :, c, :, 128 * m : 128 * (m + 1)
                ]

            def w1_tail(m):
                return w1t_s[:].rearrange("p (q m) -> p q m", q=2)[
                    :, :, 128 * m : 128 * (m + 1)
                ]

            def relu2_vec(out8, psum, m, w=512, eng=None):
                # s2' = relu(psum + 8*b2) in fp8 (scales folded)
                eng = eng or nc.vector
                t = smpool.tile([128, 512], F32, tag="r2t", name="r2t")
                eng.scalar_tensor_tensor(
                    out=t[:, 0:w], in0=psum, scalar=1.0,
                    in1=b2_s[:, m : m + 1].broadcast_to([128, w]),
                    op0=MUL, op1=ADD,
                )
                eng.tensor_scalar_max(out8, t[:, 0:w], 0.0)

            # ---- event path (cols: 64 real b + 64 junk) ----
            s1e = cpool.tile([128, 4 * 128], F8)
            for m in range(4):
                pe = pmisc.tile([128, 128], F32, tag="pmisc", name="pe")
                for c in range(C):
                    nc.tensor.matmul(
                        pe[:], w1_main(c, m), xmain(xe[:], c),
                        start=(c == 0), stop=False, perf_mode=DR,
                    )
                nc.tensor.matmul(
                    pe[:], w1_tail(m),
                    taile_s[:].rearrange("p (q i) -> p q i", q=2),
                    start=False, stop=True, perf_mode=DR,
                )
                # s1' = 8*relu(h1) = relu(psum/2 + 8*b1)
                nc.scalar.activation(
                    s1e[:, 128 * m : 128 * (m + 1)], pe[:], AF.Relu,
                    bias=b1_s[:, m : m + 1], scale=0.5,
                )

            eh28 = cpool.tile([128, 2 * 128], F8)
            for m in range(2):
                pe2 = pmisc.tile([128, 128], F32, tag="pmisc", name="pe2")
                for j in range(2):
                    nc.tensor.matmul(
                        pe2[:],
                        w2q_s[:].rearrange("p (u m) -> p u m", u=4)[
                            :, 2 * j : 2 * j + 2, 128 * m : 128 * (m + 1)
                        ],
                        s1e[:].rearrange("p (u i) -> p u i", u=4)[:, 2 * j : 2 * j + 2, :],
                        start=(j == 0), stop=(j == 1), perf_mode=DR,
                    )
                relu2_vec(eh28[:, 128 * m : 128 * (m + 1)], pe2[:], m, w=128)

            # variance pre-activation (component 1); chain runs in end phase
            pv = pmisc.tile([16, 128], F32, tag="pmisc", name="pv")
            nc.tensor.matmul(
                pv[:],
                wvm_s[:].rearrange("p (q m) -> p q m", q=2),
                xmain(xe[:], 1),
                start=True, stop=False, perf_mode=DR,
            )
            nc.tensor.matmul(
                pv[:],
                wvt_s[:].rearrange("p (q m) -> p q m", q=2),
                taile_s[:].rearrange("p (q i) -> p q i", q=2),
                start=False, stop=True, perf_mode=DR,
            )
            pvs = cpool.tile([9, BC], F32)
            nc.vector.tensor_copy(pvs[:], pv[0:9, 0:BC])

            # |e|^2 row
            esq8 = cpool.tile([128, 2 * 128], F8)
            nc.vector.tensor_mul(esq8[:], eh28[:], eh28[:])
            pne = pmisc.tile([16, 128], F32, tag="pmisc", name="pne")
            nc.tensor.matmul(
                pne[:], ones8[:].rearrange("p (q m) -> p q m", q=2),
                esq8[:].rearrange("p (u i) -> p u i", u=2),
                start=True, stop=True, perf_mode=DR,
            )
            ne2p = cpool.tile([32, BC], F32)
            nc.vector.memset(ne2p[:], 0.0)
            nc.scalar.copy(ne2p[0:1, :], pne[0:1, 0:BC])
            ne2b = cpool.tile([BC, 32], F32)
            nc.vector.transpose(ne2b[0:32, :], ne2p[:, 0:32])
            nc.vector.transpose(ne2b[32:64, :], ne2p[:, 32:64])

            # variance chain: var = softplus(pv/16 + bv); dist = exp(ndsq/var)
            ez_s = smpool.tile([9, BC], F32, tag="ez")
            nc.scalar.activation(ez_s[:], pvs[:], AF.Exp, bias=bv_s[:], scale=1.0 / XS)
            ez1_s = smpool.tile([9, BC], F32, tag="ez1")
            nc.vector.tensor_scalar_add(ez1_s[:], ez_s[:], 1.0)
            varb = smpool.tile([9, BC], F32, tag="varb")
            nc.scalar.activation(varb[:], ez1_s[:], AF.Ln)
            rv_s = smpool.tile([9, BC], F32, tag="rv")
            nc.vector.reciprocal(rv_s[:], varb[:])
            q_s = smpool.tile([9, BC], F32, tag="q")
            nc.vector.tensor_mul(q_s[:], ndsq_s[:], rv_s[:])
            qp_s = cpool.tile([32, BC], F32)
            nc.vector.memset(qp_s[:], 0.0)
            nc.scalar.activation(qp_s[0:9, :], q_s[:], AF.Exp)
            qb_s = cpool.tile([BC, 32], F32)
            nc.vector.transpose(qb_s[0:32, :], qp_s[:, 0:32])
            nc.vector.transpose(qb_s[32:64, :], qp_s[:, 32:64])
            eps_s = cpool.tile([BC, 1], F32)
            nc.vector.memset(eps_s[:], 1e-20)
            featall = cpool.tile([BC, 48], F32)
            nc.vector.memset(featall[:], 0.0)
            nc.vector.tensor_copy(featall[:, 0:NF], featd_s[:])
            nc.vector.tensor_copy(featall[:, 16:48], qb_s[:])

            # full-size tail work tiles; each half computes on its own
            # partition slice so in/out partitions stay aligned
            traw64 = cpool.tile([BC, N], F32)
            ncsq64 = cpool.tile([BC, N], F32)
            prodn = cpool.tile([BC, N], F32)
            lnp = cpool.tile([BC, N], F32)
            nrmf = cpool.tile([BC, N], F32)
            trans = cpool.tile([BC, N], F32)
            yk = cpool.tile([BC, NK * N], F32)
            ekb = cpool.tile([BC, NK * N], F32)
            pooled = cpool.tile([BC, NK], F32)
            kpc = cpool.tile([BC, NK], F32)
            kpl = cpool.tile([BC, NK], F32)
            kpw = cpool.tile([BC, NK], F32)
            kps = cpool.tile([BC, 1], F32)
            fw = cpool.tile([BC, 48], F32)
            fs = cpool.tile([BC, 1], F32)
            tot = cpool.tile([BC, 1], F32)
            emx = cpool.tile([BC, 1], F32)
            emx1 = cpool.tile([BC, 1], F32)
            outs = cpool.tile([BC, 1], F32)

            def emit_tail():
                for h in range(2):
                    HB = BC // 2
                    bs = slice(HB * h, HB * (h + 1))
                    nc.sync.dma_start(
                        traw64[bs, :],
                        trd[h][:].rearrange("o (b n) -> (o b) n", b=HB),
                    )
                    nc.scalar.dma_start(
                        ncsq64[bs, :],
                        ncd[h][:].rearrange("o (b n) -> (o b) n", b=HB),
                    )
                nc.vector.tensor_tensor(
                    out=prodn[:], in0=ncsq64[:],
                    in1=ne2b[:, 0:1].broadcast_to([BC, N]), op=MUL,
                )
                nc.scalar.activation(lnp[:], prodn[:], AF.Ln, bias=eps_s[:])
                nc.scalar.activation(nrmf[:], lnp[:], AF.Exp, scale=-0.5)
                nc.vector.tensor_mul(trans[:], traw64[:], nrmf[:])
                for k in range(NK):
                    sck = 1.0 / (SIGMAS[k] * 2.0 ** 0.5)
                    nc.scalar.activation(
                        yk[:, N * k : N * (k + 1)], trans[:], AF.Square,
                        bias=kpb_s[:, k : k + 1], scale=sck,
                    )
                nc.vector.tensor_scalar_min(yk[:], yk[:], 87.0)
                nc.scalar.activation(ekb[:], yk[:], AF.Exp, scale=-1.0)
                nc.vector.reduce_sum(
                    out=pooled[:],
                    in_=ekb[:].rearrange("b (k n) -> b k n", k=NK),
                    axis=mybir.AxisListType.X,
                )
                nc.vector.tensor_scalar_max(kpc[:], pooled[:], 1e-10)
                nc.scalar.activation(kpl[:], kpc[:], AF.Ln)
                nc.vector.tensor_mul(kpw[:], kpl[:], wkb_s[:])
                nc.vector.reduce_sum(
                    out=kps[:], in_=kpw[:], axis=mybir.AxisListType.X
                )
                nc.vector.tensor_mul(fw[:], featall[:], wcb_s[:])
                nc.vector.reduce_sum(
                    out=fs[:], in_=fw[:], axis=mybir.AxisListType.X
                )
                nc.vector.tensor_add(tot[:], fs[:], kps[:])
                nc.scalar.activation(
                    emx[:], tot[:], AF.Exp, bias=bc_s[:], scale=-1.0
                )
                nc.vector.tensor_scalar_add(emx1[:], emx[:], 1.0)
                nc.vector.reciprocal(outs[:], emx1[:])

            # ---- context groups ----
            for g in range(GROUPS):
                xt = xts[g]

                def xmain_g(c):
                    return xt[:].rearrange(
                        "p (z i q) -> p q z i", z=SUBT, q=2
                    )[:, :, :, 128 * c : 128 * (c + 1)]

                s1 = s1pool.tile([128, 4 * 512], F8, tag="s1", name=f"s1_{g}")
                for m in range(4):
                    p1 = pm1.tile([128, 512], F32)
                    for c in range(C):
                        nc.tensor.matmul(
                            p1[:], w1_main(c, m), xmain_g(c),
                            start=(c == 0), stop=False, perf_mode=DR,
                        )
                    nc.tensor.matmul(
                        p1[:], w1_tail(m),
                        tailc_s[:].rearrange("p (q i) -> p q i", q=2)[
                            :, :, 512 * g : 512 * (g + 1)
                        ],
                        start=False, stop=True, perf_mode=DR,
                    )
                    nc.scalar.activation(
                        s1[:, 512 * m : 512 * (m + 1)], p1[:], AF.Relu,
                        bias=b1_s[:, m : m + 1], scale=0.5,
                    )

                s28 = s2pool.tile([128, 2 * 512], F8, tag="s28", name=f"s28_{g}")
                for m in range(2):
                    p2 = pm2.tile([128, 512], F32)
                    for j in range(2):
                        nc.tensor.matmul(
                            p2[:],
                            w2q_s[:].rearrange("p (u m) -> p u m", u=4)[
                                :, 2 * j : 2 * j + 2, 128 * m : 128 * (m + 1)
                            ],
                            s1[:].rearrange("p (u i) -> p u i", u=4)[
                                :, 2 * j : 2 * j + 2, :
                            ],
                            start=(j == 0), stop=(j == 1), perf_mode=DR,
                        )
                    relu2_vec(s28[:, 512 * m : 512 * (m + 1)], p2[:], m)

                csq8 = csqpool.tile([128, 2 * 512], F8, tag="csq8", name=f"csq8_{g}")
                nc.vector.tensor_mul(csq8[:], s28[:], s28[:])

                s28v = s28[:].rearrange("p (u i) -> p u i", u=2)
                pT = ptn.tile([16, 512], F32, tag="pT", name="pT")
                pN = ptn.tile([16, 512], F32, tag="pN", name="pN")
                for z in range(SUBT):
                    b = SUBT * g + z
                    nc.tensor.matmul(
                        pT[0:16, 128 * z : 128 * (z + 1)],
                        eh28[:].rearrange("p (u i) -> p u i", u=2)[:, :, b : b + 16],
                        s28v[:, :, 128 * z : 128 * (z + 1)],
                        start=True, stop=True, perf_mode=DR,
                    )
                nc.tensor.matmul(
                    pN[:], ones8[:].rearrange("p (q m) -> p q m", q=2),
                    csq8[:].rearrange("p (u i) -> p u i", u=2),
                    start=True, stop=True, perf_mode=DR,
                )
                trow = smpool.tile([1, 512], F32, tag="trow", name=f"trow_{g}")
                nc.scalar.copy(trow[:], pT[0:1, :])
                nrow = smpool.tile([1, 512], F32, tag="nrow", name=f"nrow_{g}")
                nc.vector.tensor_copy(nrow[:], pN[0:1, :])
                gh, gr = divmod(g, GROUPS // 2)
                nc.sync.dma_start(trd[gh][:, 512 * gr : 512 * (gr + 1)], trow[:])
                nc.scalar.dma_start(ncd[gh][:, 512 * gr : 512 * (gr + 1)], nrow[:])
            emit_tail()
            nc.sync.dma_start(out_d.ap(), outs[:])

    nc.compile()

    # Spread SWDGE gathers across the 4 queues (ucode locks each DMASW
    # semaphore lane to one queue; lanes are assigned round-robin in
    # scheduled order, so derive queue from the assigned lane post-compile).
    import re as _re
    for blk in nc.m.functions[0].blocks:
        for inst in blk.instructions:
            if type(inst).__name__ == "InstDMAGatherAnt":
                for u in inst.sync_info.on_update:
                    m = _re.match(r"DMASW(\d+)_", u.ant_name or "")
                    if m:
                        inst.queue_num = int(m.group(1)) % 4
                        break

    _PROGRAM_CACHE["fast"] = nc
    return nc


def _wrap16(flat_idx):
    """int16 index list -> (128, n/16) tile layout: unwrapped[i] =
    tile[i % 16, i // 16], replicated into all 8 16-partition stripes."""
    n = flat_idx.shape[0]
    t = np.zeros((16, n // 16), np.int16)
    t[np.arange(n) % 16, np.arange(n) // 16] = flat_idx
    return np.tile(t, (8, 1))


def _pack_dr_k(mat, rows):
    """[K, N] -> [128, 2, N] fp8 DoubleRow K-pair layout, zero-padded."""
    k, n = mat.shape
    assert k <= rows <= 256
    out = np.zeros((256, n), np.float32)
    out[:k] = mat
    return np.ascontiguousarray(
        out.reshape(2, 128, n).transpose(1, 0, 2).reshape(128, 2 * n)
    )


def _prep_fast_consts(inputs):
    """Shared (core-independent) fp8 weight re-layouts."""
    f8 = ml_dtypes.float8_e4m3fn
    W1 = np.asarray(inputs["W1"], np.float32)   # (H1, C*E)
    W2 = np.asarray(inputs["W2"], np.float32)   # (H2, H1)
    Wv = np.asarray(inputs["Wv"], np.float32)   # (9, E)
    b1 = np.asarray(inputs["b1"], np.float32)
    b2 = np.asarray(inputs["b2"], np.float32)
    bv = np.asarray(inputs["bv"], np.float32)

    W1q = W1.astype(f8).astype(np.float32)      # quantize once, reuse
    Wvq = Wv.astype(f8).astype(np.float32)

    # main: w1m[p, c, q, m] = W1q[m, E*c + 2p+q]  (elems < 256)
    w1m = np.zeros((128, C, 2, H1), np.float32)
    for c in range(C):
        blk = W1q[:, E * c : E * c + EM]        # (H1, 256)
        w1m[:, c, :, :] = blk.T.reshape(128, 2, H1)
    # tail: k_t = c*44 + (e-256); w1t[p, q, m] = W1q[m, ktmap(q*128+p)]
    tail = np.zeros((256, H1), np.float32)
    for c in range(C):
        tail[ET * c : ET * (c + 1)] = W1q[:, E * c + EM : E * (c + 1)].T
    w1t = tail.reshape(2, 128, H1).transpose(1, 0, 2)

    # w2: [p, u, m] = W2q[m, 128u + p]
    W2q = W2.astype(f8).astype(np.float32)
    w2q = W2q.T.reshape(4, 128, H2).transpose(1, 0, 2)

    # wv main (component 1 elems < 256) and tail rows 44..87; M padded to 16
    wvm_full = np.zeros((256, 16), np.float32)
    wvm_full[:, 0:9] = Wvq[:, :EM].T
    wvm = wvm_full.reshape(2, 128, 16).transpose(1, 0, 2)
    wvt_full = np.zeros((256, 16), np.float32)
    wvt_full[ET : 2 * ET, 0:9] = Wvq[:, EM:E].T
    wvt = wvt_full.reshape(2, 128, 16).transpose(1, 0, 2)

    mus = np.array(MUS, np.float32)
    sig = np.array(SIGMAS, np.float32)
    kpb = np.tile((-mus / (sig * np.sqrt(2.0)))[None, :], (BC, 1))

    return {
        "w1m": np.ascontiguousarray(w1m.reshape(128, C * 2 * H1)).astype(f8),
        "w1t": np.ascontiguousarray(w1t.reshape(128, 2 * H1)).astype(f8),
        "w2q": np.ascontiguousarray(w2q.reshape(128, 4 * H2)).astype(f8),
        "wvm": np.ascontiguousarray(wvm.reshape(128, 2 * 16)).astype(f8),
        "wvt": np.ascontiguousarray(wvt.reshape(128, 2 * 16)).astype(f8),
        "b1d": np.ascontiguousarray(8.0 * b1.reshape(4, 128).T),
        "b2d": np.ascontiguousarray(8.0 * b2.reshape(2, 128).T),
        "bvd": bv.reshape(9, 1),
        "kpb": np.ascontiguousarray(kpb.astype(np.float32)),
    }


def _prep_fast_core(inputs, consts, tableq, core):
    """Per-core shard prep for the fast fp8 path."""
    f8 = ml_dtypes.float8_e4m3fn
    Wc = np.asarray(inputs["Wc"], np.float32)
    bc = np.asarray(inputs["bc"], np.float32)

    sl = slice(core * BC, (core + 1) * BC)
    ev = np.asarray(inputs["batch_event"][sl], np.int64)          # (BC, C)
    feats = np.asarray(inputs["batch_features"][sl], np.float32)  # (BC, NF)
    dists = np.asarray(inputs["batch_distances"][sl], np.float32) # (BC, 9)
    ctx = np.asarray(inputs["batch_context"][sl], np.int64)       # (BC, N, C)

    allidx = np.concatenate([ctx.reshape(-1), ev.reshape(-1)])
    uniq, inv = np.unique(allidx, return_inverse=True)
    assert len(uniq) <= CT
    tq = tableq[uniq]                                   # (U, E) fp8
    ctab = np.zeros((CT, EM), f8)
    ctab[: len(uniq)] = tq[:, :EM]
    rctx = inv[: ctx.size].astype(np.int64).reshape(BC, N, C)
    rev = inv[ctx.size :].astype(np.int64).reshape(BC, C)

    # context gather idx: per (g, s): 512 idx with i = c*128 + n
    ci = rctx.reshape(GROUPS, SUBT, N, C).transpose(0, 1, 3, 2)  # g,s,c,n
    cidx = np.concatenate(
        [
            _wrap16(ci[g, s].reshape(-1).astype(np.int16))
            for g in range(GROUPS)
            for s in range(SUBT)
        ],
        axis=1,
    )
    # event idx: i = c*128 + b; b >= BC -> row 0 junk
    ei = np.zeros((C, 128), np.int16)
    ei[:, :BC] = rev.T.astype(np.int16)

    # tails: [k_t = c*44 + e', col = 128b + n] from the SAME quantized table
    tl = tq[:, EM:E].astype(np.float32)                 # (U, 44)
    tailc = tl[rctx]                                    # (BC, N, C, 44)
    tailc = tailc.transpose(2, 3, 0, 1).reshape(KT, BN)
    taile = tl[rev].transpose(1, 2, 0).reshape(KT, BC)  # (176, 64)
    taile = np.concatenate([taile, np.zeros((KT, 128 - BC), np.float32)], axis=1)

    # per-b Wc rows: cols 0..7 features, 16..24 dist_emb
    wc_row = np.zeros((48,), np.float32)
    wc_row[0:NF] = Wc[0, 9 : 9 + NF]
    wc_row[16 : 16 + 9] = Wc[0, 0:9]
    wkp = (Wc[0, NF + 9 :] * 0.01).astype(np.float32)

    m = dict(consts)
    m.update(
        {
            "ctab": ctab,
            "tailc": _pack_dr_k(tailc, KT).astype(f8),
            "taile": _pack_dr_k(taile, KT).astype(f8),
            "cidx": np.ascontiguousarray(cidx),
            "eidx": np.ascontiguousarray(_wrap16(ei.reshape(-1))),
            "wcb": np.tile(wc_row, (BC, 1)),
            "wkb": np.tile(wkp, (BC, 1)),
            "bcd": np.full((BC, 1), -float(bc[0]), np.float32),
            "ndsq": np.ascontiguousarray(-(dists * dists).T),
            "featd": np.ascontiguousarray(feats),
        }
    )
    return m


def _numpy_fallback(inputs):
    """Exact reference math on host (safety net for >32K unique rows)."""
    t = np.asarray(inputs["event_table"], np.float32)
    W1 = np.asarray(inputs["W1"], np.float32)
    b1 = np.asarray(inputs["b1"], np.float32)
    W2 = np.asarray(inputs["W2"], np.float32)
    b2 = np.asarray(inputs["b2"], np.float32)
    Wv = np.asarray(inputs["Wv"], np.float32)
    bv = np.asarray(inputs["bv"], np.float32)
    Wc = np.asarray(inputs["Wc"], np.float32)
    bc = np.asarray(inputs["bc"], np.float32)
    ev = np.asarray(inputs["batch_event"], np.int64)
    feats = np.asarray(inputs["batch_features"], np.float32)
    dists = np.asarray(inputs["batch_distances"], np.float32)
    ctx = np.asarray(inputs["batch_context"], np.int64)

    def mlp(x):
        x = np.maximum(x @ W1.T + b1, 0.0)
        return np.maximum(x @ W2.T + b2, 0.0)

    def l2n(x):
        n = np.linalg.norm(x, axis=-1, keepdims=True)
        return x / np.maximum(n, 1e-12)

    ee = t[ev]                                    # (B, C, E)
    ce = t[ctx]                                   # (B, N, C, E)
    var = np.log1p(np.exp(ee[:, 1, :] @ Wv.T + bv))
    de = np.exp(-(dists * dists) / var)
    extracted = np.concatenate([de, feats], axis=1)
    er = mlp(ee.reshape(B, 1, C * E))
    cr = mlp(ce.reshape(B, N, C * E))
    trans = np.einsum("bmd,bnd->bmn", l2n(er), l2n(cr))
    mus = np.array(MUS, np.float32)
    sig = np.array(SIGMAS, np.float32)
    kk = np.exp(-((trans[..., None] - mus) ** 2) / (2.0 * sig**2))
    kp = np.log(np.clip(kk.sum(axis=2), 1e-10, None)) * 0.01
    allf = np.concatenate([extracted[:, None, :], kp], axis=-1)
    scores = (allf @ Wc.T + bc)[..., 0]
    return (1.0 / (1.0 + np.exp(-scores))).astype(np.float32)


def kernel(**inputs) -> np.ndarray:
    ctx = np.asarray(inputs["batch_context"], np.int64)
    ev = np.asarray(inputs["batch_event"], np.int64)
    fast = True
    for core in range(NCORES):
        sl = slice(core * BC, (core + 1) * BC)
        nuniq = len(np.unique(np.concatenate(
            [ctx[sl].reshape(-1), ev[sl].reshape(-1)])))
        if nuniq > CT:
            fast = False
            break
    if not fast:  # pragma: no cover - impossible for random inputs
        return _numpy_fallback(inputs)

    f8 = ml_dtypes.float8_e4m3fn
    tableq = (np.asarray(inputs["event_table"], np.float32) * XS).astype(f8)
    consts = _prep_fast_consts(inputs)
    nc = _build_fast()
    in_maps = [
        _prep_fast_core(inputs, consts, tableq, core) for core in range(NCORES)
    ]
    res = run_bass_kernel_spmd(nc, in_maps, core_ids=list(range(NCORES)))
    return np.concatenate([r["out"] for r in res.results], axis=0)


if __name__ == "__main__":
    nc = _build_fast()
    print("program built ok")
